# revision 24
# baseline (speedup 1.0000x reference)
"""HSTU-style dense transformer for sequence modeling on 8 Trainium2 NeuronCores.

Sharding: data-parallel over batch (B=8 -> 1 sequence per core). All weights
replicated, stored in DRAM as fp16 (halves HBM traffic; TRN2 PE runs fp16 at
the same 1 cycle/row as f32r). Activations are kept feature-major
[D=partitions, T=free]; the residual stream stays fp32, while the normalized
stream / attention operands / GEMM inputs are fp16 (PSUM accumulation is fp32).

LayerNorm stats are broadcast-reduced with a ones[128,128] matmul so the whole
mean/rsqrt chain runs on [128,S] tiles and never round-trips the PE for a
separate broadcast. Attention computes transposed scores [kt, qt]; sigmoid is
one activation per chunk with an in-place causal-mask multiply on the diagonal
block; the AV matmuls for the two heads of a pair write one PSUM tile at
partition offsets 0/64 (col-tiled, concurrent on the PE array).

Host side only marshals: embedding gather + positional add, weight pre-tiling
into DMA-contiguous fp16 layouts, and the final [V,T] -> [S,V] untranspose.
"""

import sys

sys.path.insert(0, "/opt/trn_rl_repo")

import numpy as np
import ml_dtypes

import concourse.bass as bass  # noqa: F401  (keeps bass registered before bacc)
import concourse.tile as tile
from concourse import bacc, mybir
from concourse.bass import ts
from concourse.bass_utils import run_bass_kernel_spmd

B, S, D, H, L, V = 8, 512, 1024, 16, 6, 32000
DH = D // H
LN_EPS = 1e-5
N_CORES = 8
NC_D = D // 128      # 8 feature chunks
NC_T = S // 128      # 4 token chunks
NC_V = V // 128      # 250 vocab chunks
NP = 8               # head pairs

F32 = mybir.dt.float32
F32R = mybir.dt.float32r
F16 = mybir.dt.float16
AF = mybir.ActivationFunctionType
OP = mybir.AluOpType
NPF16 = np.float16

_prog_cache = {}


def _build(cfg):
    """Build + compile the SPMD per-core program. cfg is a hashable tuple."""
    (use_lng, use_lnb, use_bqk, use_bv, use_bg, use_bo, use_bp, rpb_nz) = cfg

    nc = bacc.Bacc("TRN2", target_bir_lowering=False, debug=False)

    x0_d = nc.dram_tensor("x0t", [NC_D, 128, S], F32R, kind="ExternalInput").ap()
    x0b_d = nc.dram_tensor("x0b", [NC_D, 128, S], F16, kind="ExternalInput").ap()
    wqk_d = nc.dram_tensor("wqk", [L, 16, 128, 1024], F16, kind="ExternalInput").ap()
    wv_d = nc.dram_tensor("wv", [L, 16, 128, 512], F16, kind="ExternalInput").ap()
    wg_d = nc.dram_tensor("wg", [L, NC_D, 128, 1024], F16, kind="ExternalInput").ap()
    wo_d = nc.dram_tensor("wo", [L, NC_D, 128, 1024], F16, kind="ExternalInput").ap()
    wp_d = nc.dram_tensor("wp", [NC_V, 128, 1024], F16, kind="ExternalInput").ap()
    tri_d = nc.dram_tensor("tri", [128, 128], F16, kind="ExternalInput").ap()
    ones_d = nc.dram_tensor("onesq", [128, 128], F16, kind="ExternalInput").ap()
    need_oner = rpb_nz or use_bv
    oner_d = nc.dram_tensor("oner", [1, 128], F32R, kind="ExternalInput").ap() if need_oner else None
    lng_d = nc.dram_tensor("lng", [L, NC_D, 128], F32, kind="ExternalInput").ap() if use_lng else None
    lnb_d = nc.dram_tensor("lnb", [L, NC_D, 128], F32, kind="ExternalInput").ap() if use_lnb else None
    bqk_d = nc.dram_tensor("bqk", [L, 16, 128], F32, kind="ExternalInput").ap() if use_bqk else None
    bv_d = nc.dram_tensor("bv", [L, 2, 1, 512], F32R, kind="ExternalInput").ap() if use_bv else None
    bg_d = nc.dram_tensor("bg", [L, NC_D, 128], F32, kind="ExternalInput").ap() if use_bg else None
    bo_d = nc.dram_tensor("bo", [L, NC_D, 128], F32, kind="ExternalInput").ap() if use_bo else None
    bp_d = nc.dram_tensor("bp", [NC_V, 128], F32, kind="ExternalInput").ap() if use_bp else None
    rpb_d = nc.dram_tensor("rpb", [1, H * L], F32R, kind="ExternalInput").ap() if rpb_nz else None
    out_d = nc.dram_tensor("logits_t", [NC_V, 128, S], F16, kind="ExternalOutput").ap()

    with tile.TileContext(nc) as tc, nc.allow_low_precision(
        reason="fp16 GEMM operands; accumulation stays fp32 in PSUM"
    ):
        from contextlib import ExitStack

        with ExitStack() as ctx:
            cp = ctx.enter_context(tc.tile_pool(name="consts", bufs=1))
            xp = ctx.enter_context(tc.tile_pool(name="x", bufs=2))
            xbp = ctx.enter_context(tc.tile_pool(name="xbf", bufs=2))
            xnp = ctx.enter_context(tc.tile_pool(name="xn", bufs=2))
            up = ctx.enter_context(tc.tile_pool(name="u", bufs=1))
            vp = ctx.enter_context(tc.tile_pool(name="v", bufs=1))
            gp = ctx.enter_context(tc.tile_pool(name="g", bufs=1))
            qkp = ctx.enter_context(tc.tile_pool(name="qk", bufs=3))
            dmp = ctx.enter_context(tc.tile_pool(name="dmy", bufs=2))
            tmp = ctx.enter_context(tc.tile_pool(name="tmp", bufs=3))
            bp_pool = ctx.enter_context(tc.tile_pool(name="bcast", bufs=2))
            atp = ctx.enter_context(tc.tile_pool(name="at", bufs=4))
            wbp = ctx.enter_context(tc.tile_pool(name="wb", bufs=8))
            wvp = ctx.enter_context(tc.tile_pool(name="wvp", bufs=2))
            op_pool = ctx.enter_context(tc.tile_pool(name="out", bufs=4))
            prm = ctx.enter_context(tc.tile_pool(name="prm", bufs=2))
            pmm = ctx.enter_context(tc.tile_pool(name="pmm", bufs=2, space="PSUM"))
            pao = ctx.enter_context(tc.tile_pool(name="pao", bufs=2, space="PSUM"))
            psc = ctx.enter_context(tc.tile_pool(name="psc", bufs=4, space="PSUM"))

            mm = nc.tensor.matmul

            ones_t = cp.tile([128, 128], F16)
            nc.sync.dma_start(ones_t[:], ones_d[:])
            tri_t = cp.tile([128, 128], F16)
            nc.sync.dma_start(tri_t[:], tri_d[:])
            eps_t = cp.tile([128, 1], F32)
            nc.vector.memset(eps_t[:], LN_EPS)
            if need_oner:
                oner = cp.tile([1, 128], F32R)
                nc.sync.dma_start(oner[:], oner_d[:])
            if rpb_nz:
                rpb_row = cp.tile([1, H * L], F32R)
                nc.sync.dma_start(rpb_row[:], rpb_d[:])
                # broadcast to [128, H*L] so column slices give per-partition bias
                prb = psc.tile([128, 512], F32, tag="sc")
                mm(prb[:, : H * L], oner[:], rpb_row[:], start=True, stop=True)
                rpb_t = cp.tile([128, H * L], F32)
                nc.scalar.copy(rpb_t[:], prb[:, : H * L])
            if use_bp:
                bp_t = cp.tile([128, NC_V], F32)
                nc.sync.dma_start(bp_t[:], bp_d.rearrange("v p -> p v"))

            x_cur = xp.tile([128, NC_D * S], F32R, tag="x")
            xb_cur = xbp.tile([128, NC_D * S], F16, tag="xb")
            for c in range(NC_D):
                nc.sync.dma_start(xb_cur[:, ts(c, S)], x0b_d[c])
            for c in range(NC_D):
                nc.sync.dma_start(x_cur[:, ts(c, S)], x0_d[c])

            def act_preload(func):
                # dummy activation: forces the ScalarE function-table load to
                # happen while ACT is otherwise idle, off the critical chain
                dmy = dmp.tile([128, 1], F16, tag="dmy")
                nc.scalar.activation(dmy[:], eps_t[:, 0:1], func, bias=0.0, scale=1.0)

            act_preload(AF.Abs_reciprocal_sqrt)

            for l in range(L):
                # ---- per-layer params ----
                if use_lng:
                    lng_t = prm.tile([128, NC_D], F32, tag="lng")
                    nc.sync.dma_start(lng_t[:], lng_d[l].rearrange("c p -> p c"))
                if use_lnb:
                    lnb_t = prm.tile([128, NC_D], F32, tag="lnb")
                    nc.sync.dma_start(lnb_t[:], lnb_d[l].rearrange("c p -> p c"))
                if use_bqk:
                    bqk_t = prm.tile([128, 16], F32, tag="bqk")
                    nc.sync.dma_start(bqk_t[:], bqk_d[l].rearrange("c p -> p c"))
                if use_bg:
                    bg_t = prm.tile([128, NC_D], F32, tag="bg")
                    nc.sync.dma_start(bg_t[:], bg_d[l].rearrange("c p -> p c"))
                if use_bo:
                    bo_t = prm.tile([128, NC_D], F32, tag="bo")
                    nc.sync.dma_start(bo_t[:], bo_d[l].rearrange("c p -> p c"))

                # ---- LayerNorm stats from the fp16 shadow, broadcast-reduced
                # to [128, S]; the whole chain runs on fp16 tiles (DVE 2x) ----
                ps_s = psc.tile([128, S], F32, tag="sc")
                ps_q = psc.tile([128, S], F32, tag="sc")
                for c in range(NC_D):
                    xc = xb_cur[:, ts(c, S)]
                    mm(ps_s[:], ones_t[:], xc, start=(c == 0), stop=(c == NC_D - 1))
                    sq = tmp.tile([128, S], F16, tag="sqb")
                    nc.vector.tensor_mul(sq[:], xc, xc)
                    mm(ps_q[:], ones_t[:], sq[:], start=(c == 0), stop=(c == NC_D - 1))
                mu_b = bp_pool.tile([128, S], F16, tag="mu")
                nc.vector.tensor_scalar_mul(mu_b[:], ps_s[:], 1.0 / D)
                musq = tmp.tile([128, S], F16, tag="sqb")
                nc.vector.tensor_mul(musq[:], mu_b[:], mu_b[:])
                var = tmp.tile([128, S], F32, tag="tmp")
                nc.vector.scalar_tensor_tensor(
                    var[:], ps_q[:], 1.0 / D, musq[:], op0=OP.mult, op1=OP.subtract
                )
                rs_b = bp_pool.tile([128, S], F16, tag="rs")
                nc.scalar.activation(rs_b[:], var[:], AF.Abs_reciprocal_sqrt,
                                     bias=eps_t[:, 0:1], scale=1.0)
                act_preload(AF.Silu)

                xn = xnp.tile([128, NC_D * S], F16, tag="xn")
                for c in range(NC_D):
                    t = tmp.tile([128, S], F16, tag="sqb")
                    nc.vector.tensor_sub(t[:], xb_cur[:, ts(c, S)], mu_b[:])
                    if use_lng or use_lnb:
                        t2 = tmp.tile([128, S], F32, tag="tmp")
                        nc.vector.tensor_mul(t2[:], t[:], rs_b[:])
                        nc.scalar.activation(
                            xn[:, ts(c, S)], t2[:], AF.Identity,
                            bias=(lnb_t[:, c : c + 1] if use_lnb else 0.0),
                            scale=(lng_t[:, c : c + 1] if use_lng else 1.0),
                        )
                    else:
                        nc.vector.tensor_mul(xn[:, ts(c, S)], t[:], rs_b[:])

                # ---- qk projection for pairs 0/1, four PSUM banks wide: each
                # xn chunk unlocks 4 matmuls, so the PE streams through the
                # LayerNorm chunk-production window without stalling ----
                def qk_copy(dst, pq, fidx):
                    if use_bqk:
                        nc.scalar.activation(
                            dst[:], pq[:], AF.Identity,
                            bias=bqk_t[:, fidx : fidx + 1], scale=1.0,
                        )
                    else:
                        nc.vector.tensor_copy(dst[:], pq[:])

                qk4_f = (0, 8, 1, 9)  # q0, k0, q1, k1
                qk4_w = []
                for fidx in qk4_f:
                    wt = wbp.tile([128, 1024], F16, tag="wb")
                    nc.sync.dma_start(wt[:], wqk_d[l, fidx])
                    qk4_w.append(wt)
                qk4_p = [pmm.tile([128, S], F32, tag="mm", name="qk4p0"),
                         pmm.tile([128, S], F32, tag="mm", name="qk4p1"),
                         pao.tile([128, S], F32, tag="ao", name="qk4p2"),
                         pao.tile([128, S], F32, tag="ao", name="qk4p3")]
                for fi in range(NC_D):
                    for j in range(4):
                        mm(qk4_p[j][:], qk4_w[j][:, ts(fi, 128)], xn[:, ts(fi, S)],
                           start=(fi == 0), stop=(fi == NC_D - 1))
                qk01 = []
                for j, fidx in enumerate(qk4_f):
                    dst = qkp.tile([128, S], F16, tag=("qc" if fidx < 8 else "kc"))
                    qk_copy(dst, qk4_p[j], fidx)
                    qk01.append(dst)

                # ---- V projection, token-major [t, fo] (xn stationary, wv moving) ----
                wv_t = wvp.tile([128, 16 * 512], F16, tag="wv")
                for i in range(16):
                    nc.sync.dma_start(wv_t[:, ts(i, 512)], wv_d[l, i])
                if use_bv:
                    vb = []
                    for foB in range(2):
                        pvb = psc.tile([128, 512], F32, tag="sc")
                        bvrow = cp.tile([1, 512], F32R, tag=f"bvr{foB}")
                        nc.sync.dma_start(bvrow[:], bv_d[l, foB])
                        mm(pvb[:], oner[:], bvrow[:], start=True, stop=True)
                        vbt = bp_pool.tile([128, 512], F32, tag="vb")
                        nc.scalar.copy(vbt[:], pvb[:])
                        vb.append(vbt)
                v = vp.tile([128, NC_T * 1024], F16, tag="v")
                for foB in range(2):
                    for tc_ in range(NC_T):
                        pv = pmm.tile([128, 512], F32, tag="mm")
                        for fi in range(NC_D):
                            mm(
                                pv[:],
                                xn[:, fi * S + tc_ * 128 : fi * S + tc_ * 128 + 128],
                                wv_t[:, ts(foB * 8 + fi, 512)],
                                start=(fi == 0),
                                stop=(fi == NC_D - 1),
                            )
                        dst = v[:, tc_ * 1024 + foB * 512 : tc_ * 1024 + foB * 512 + 512]
                        if use_bv:
                            nc.vector.tensor_add(dst, pv[:], vb[foB][:])
                        else:
                            nc.vector.tensor_copy(dst, pv[:])

                # ---- gate: u = silu(xn @ gate_w + bg) (feature-major) ----
                u = up.tile([128, NC_D * S], F16, tag="u")
                for f in range(NC_D):
                    wt = wbp.tile([128, 1024], F16, tag="wb")
                    nc.sync.dma_start(wt[:], wg_d[l, f])
                    pu = pmm.tile([128, S], F32, tag="mm")
                    for fi in range(NC_D):
                        mm(pu[:], wt[:, ts(fi, 128)], xn[:, ts(fi, S)],
                           start=(fi == 0), stop=(fi == NC_D - 1))
                    nc.scalar.activation(
                        u[:, ts(f, S)], pu[:], AF.Silu,
                        bias=(bg_t[:, f : f + 1] if use_bg else 0.0), scale=1.0,
                    )
                act_preload(AF.Sigmoid)

                # ---- attention, one head pair (= one q/k feature chunk) at a time;
                # the qk projection runs one pair ahead of the score/AV chain so
                # the PE never waits on the PSUM->SBUF qk copies. ----
                g = gp.tile([128, NC_D * S], F16, tag="g")

                def compute_qk(p_):
                    qc = qkp.tile([128, S], F16, tag="qc")
                    kc = qkp.tile([128, S], F16, tag="kc")
                    for (dst, fidx) in ((qc, p_), (kc, 8 + p_)):
                        wt = wbp.tile([128, 1024], F16, tag="wb")
                        nc.sync.dma_start(wt[:], wqk_d[l, fidx])
                        pq = pmm.tile([128, S], F32, tag="mm")
                        for fi in range(NC_D):
                            mm(pq[:], wt[:, ts(fi, 128)], xn[:, ts(fi, S)],
                               start=(fi == 0), stop=(fi == NC_D - 1))
                        qk_copy(dst, pq, fidx)
                    return qc, kc

                def attention(p_, qc, kc):
                    # ao: one [128, S] PSUM tile; the two heads of the pair
                    # write partitions 0:64 / 64:128 (col-tiled, concurrent).
                    ao = pao.tile([128, S], F32, tag="ao")

                    # software-pipeline: scores for chunk c+1 are issued before
                    # the AV matmuls of chunk c, so the PE streams through the
                    # sigmoid/mask handoff without stalling.
                    def make_att(c):
                        n = S - 128 * c
                        atts = []
                        for (o, hh) in ((0, 2 * p_), (64, 2 * p_ + 1)):
                            sc = psc.tile([128, S], F32, tag="sc")
                            mm(sc[:, :n], kc[o : o + 64, ts(c, 128)],
                               qc[o : o + 64, c * 128 : S], start=True, stop=True)
                            sig_bias = rpb_t[:, l * H + hh : l * H + hh + 1] if rpb_nz else 0.0
                            att = atp.tile([128, S], F16, tag="at")
                            nc.scalar.activation(att[:, :n], sc[:, :n], AF.Sigmoid,
                                                 bias=sig_bias, scale=DH**-0.5)
                            nc.vector.tensor_mul(att[:, 0:128], att[:, 0:128], tri_t[:])
                            atts.append(att)
                        return atts

                    def do_av(c, atts):
                        n = S - 128 * c
                        for (o, att) in ((0, atts[0]), (64, atts[1])):
                            # two interleaved accumulation groups in one bank,
                            # partition-disjoint: start's pending-zero marking
                            # is scoped to the partitions each matmul writes.
                            mm(
                                ao[o : o + 64, c * 128 : S],
                                v[:, c * 1024 + p_ * 128 + o : c * 1024 + p_ * 128 + o + 64],
                                att[:, 0:n],
                                start=(c == 0),
                                stop=(c == NC_T - 1),
                                skip_group_check=True,
                            )

                    att_prev = make_att(0)
                    for c in range(1, NC_T):
                        atts = make_att(c)
                        do_av(c - 1, att_prev)
                        att_prev = atts
                    do_av(NC_T - 1, att_prev)

                    nc.vector.tensor_mul(g[:, ts(p_, S)], ao[:], u[:, ts(p_, S)])

                qks = {0: (qk01[0], qk01[1]), 1: (qk01[2], qk01[3])}
                for p_ in range(NP):
                    if p_ + 2 < NP:
                        qks[p_ + 2] = compute_qk(p_ + 2)
                    attention(p_, *qks.pop(p_))
                if l + 1 < L:
                    act_preload(AF.Abs_reciprocal_sqrt)

                # ---- out projection + residual (fp32) + bf16 shadow ----
                last = l == L - 1
                x_new = None if last else xp.tile([128, NC_D * S], F32R, tag="x")
                xb_new = xbp.tile([128, NC_D * S], F16, tag="xb")
                for f in range(NC_D):
                    wt = wbp.tile([128, 1024], F16, tag="wb")
                    nc.sync.dma_start(wt[:], wo_d[l, f])
                    pd = pmm.tile([128, S], F32, tag="mm")
                    for fi in range(NC_D):
                        mm(pd[:], wt[:, ts(fi, 128)], g[:, ts(fi, S)],
                           start=(fi == 0), stop=(fi == NC_D - 1))
                    src = pd[:]
                    if use_bo:
                        t3 = tmp.tile([128, S], F32, tag="tmp")
                        nc.scalar.activation(t3[:], pd[:], AF.Identity,
                                             bias=bo_t[:, f : f + 1], scale=1.0)
                        src = t3[:]
                    if last:
                        nc.vector.tensor_add(xb_new[:, ts(f, S)], src, x_cur[:, ts(f, S)])
                    else:
                        nc.vector.tensor_add(x_new[:, ts(f, S)], src, x_cur[:, ts(f, S)])
                        nc.vector.tensor_copy(xb_new[:, ts(f, S)], x_new[:, ts(f, S)])
                x_cur = x_new
                xb_cur = xb_new

            # ---- logits: [V, T] feature(vocab)-major; x is fp16 after layer L-1 ----
            for vo in range(NC_V):
                wt = wbp.tile([128, 1024], F16, tag="wb")
                nc.sync.dma_start(wt[:], wp_d[vo])
                pl = pmm.tile([128, S], F32, tag="mm")
                for fi in range(NC_D):
                    mm(pl[:], wt[:, ts(fi, 128)], xb_cur[:, ts(fi, S)],
                       start=(fi == 0), stop=(fi == NC_D - 1))
                ot = op_pool.tile([128, S], F16, tag="o")
                if use_bp:
                    nc.scalar.activation(ot[:], pl[:], AF.Identity,
                                         bias=bp_t[:, vo : vo + 1], scale=1.0)
                elif vo % 2 == 0:
                    nc.scalar.copy(ot[:], pl[:])
                else:
                    nc.vector.tensor_copy(ot[:], pl[:])
                nc.sync.dma_start(out_d[vo], ot[:])

    nc.compile()
    return nc


def _get_program(cfg):
    nc = _prog_cache.get(cfg)
    if nc is None:
        nc = _build(cfg)
        _prog_cache[cfg] = nc
    return nc


def _marshal(inputs):
    """Host-side input marshalling into DMA-friendly layouts."""
    f = np.float32
    input_ids = np.asarray(inputs["input_ids"])
    emb = np.asarray(inputs["embedding"], f)
    pos = np.asarray(inputs["pos_encoding"], f)
    qkv_w = np.asarray(inputs["qkv_w"], f)
    gate_w = np.asarray(inputs["gate_w"], f)
    out_w = np.asarray(inputs["out_w"], f)
    proj_w = np.asarray(inputs["proj_w"], f)

    x0 = emb[input_ids] + pos[:, :S, :]                       # [B, S, D]
    # feature-major per core: [D, S] -> [NC_D, 128, S]
    x0t = np.ascontiguousarray(x0.transpose(0, 2, 1)).reshape(B, NC_D, 128, S)

    # lhsT tiles [K=fi(128), M=fo(128)] packed 8-fi-wide: [l, fo, 128p, 8c*128j]
    def lhs_tiles(w, nfo):  # w: [L, D, nfo*128]
        r = w.reshape(L, NC_D, 128, nfo, 128).transpose(0, 3, 2, 1, 4)
        return np.ascontiguousarray(r.reshape(L, nfo, 128, NC_D * 128).astype(NPF16))

    wqk = lhs_tiles(qkv_w[:, :, :2048], 16)                   # [6,16,128,1024]
    wg = lhs_tiles(gate_w, NC_D)                              # [6,8,128,1024]
    wo = lhs_tiles(out_w, NC_D)                               # [6,8,128,1024]
    # v-section as moving tiles [K=fi(128), N=fo(512)]: [l, foB*8+fi, 128, 512]
    wv = qkv_w[:, :, 2048:].reshape(L, NC_D, 128, 2, 512).transpose(0, 3, 1, 2, 4)
    wv = np.ascontiguousarray(wv.reshape(L, 16, 128, 512).astype(NPF16))
    wp = proj_w.reshape(NC_D, 128, NC_V, 128).transpose(2, 1, 0, 3)
    wp = np.ascontiguousarray(wp.reshape(NC_V, 128, NC_D * 128).astype(NPF16))

    tri = np.triu(np.ones((128, 128), f)).astype(NPF16)
    onesq = np.ones((128, 128), NPF16)
    oner = np.ones((1, 128), f)

    qkv_b = np.asarray(inputs["qkv_b"], f)
    gate_b = np.asarray(inputs["gate_b"], f)
    out_b = np.asarray(inputs["out_b"], f)
    proj_b = np.asarray(inputs["proj_b"], f)
    ln_g = np.asarray(inputs["ln_g"], f)
    ln_b = np.asarray(inputs["ln_b"], f)
    rpb = np.asarray(inputs["rel_pos_bias"], f)

    use_lng = not np.all(ln_g == 1.0)
    use_lnb = np.any(ln_b != 0.0)
    use_bqk = np.any(qkv_b[:, :2048] != 0.0)
    use_bv = np.any(qkv_b[:, 2048:] != 0.0)
    use_bg = np.any(gate_b != 0.0)
    use_bo = np.any(out_b != 0.0)
    use_bp = np.any(proj_b != 0.0)
    rpb_nz = bool(np.any(rpb != 0.0))

    shared = {
        "wqk": wqk, "wv": wv, "wg": wg, "wo": wo, "wp": wp,
        "tri": tri, "onesq": onesq,
    }
    if rpb_nz or use_bv:
        shared["oner"] = oner
    if use_lng:
        shared["lng"] = np.ascontiguousarray(ln_g.reshape(L, NC_D, 128))
    if use_lnb:
        shared["lnb"] = np.ascontiguousarray(ln_b.reshape(L, NC_D, 128))
    if use_bqk:
        shared["bqk"] = np.ascontiguousarray(qkv_b[:, :2048].reshape(L, 16, 128))
    if use_bv:
        shared["bv"] = np.ascontiguousarray(qkv_b[:, 2048:].reshape(L, 2, 1, 512))
    if use_bg:
        shared["bg"] = np.ascontiguousarray(gate_b.reshape(L, NC_D, 128))
    if use_bo:
        shared["bo"] = np.ascontiguousarray(out_b.reshape(L, NC_D, 128))
    if use_bp:
        shared["bp"] = np.ascontiguousarray(proj_b.reshape(NC_V, 128))
    if rpb_nz:
        shared["rpb"] = np.ascontiguousarray(rpb.reshape(1, L * H))

    cfg = (use_lng, use_lnb, use_bqk, use_bv, use_bg, use_bo, use_bp, rpb_nz)
    in_maps = []
    for b in range(B):
        m = dict(shared)
        m["x0t"] = np.ascontiguousarray(x0t[b])
        m["x0b"] = np.ascontiguousarray(x0t[b].astype(NPF16))
        in_maps.append(m)
    return cfg, in_maps


def run(inputs, mm_mode="bf16", trace=False):
    cfg, in_maps = _marshal(inputs)
    nc = _get_program(cfg)
    res = run_bass_kernel_spmd(nc, in_maps, core_ids=list(range(N_CORES)), trace=trace)
    out = np.empty((B, S, V), np.float32)
    for b in range(B):
        lt = res.results[b]["logits_t"].astype(np.float32).reshape(V, S)
        out[b] = lt.T
    return out, res


def kernel(**inputs) -> np.ndarray:
    out, _ = run(inputs)
    return out


# revision 30
# speedup vs baseline: 1.0846x; 1.0846x over previous
"""HSTU-style dense transformer for sequence modeling on 8 Trainium2 NeuronCores.

Sharding: data-parallel over batch (B=8 -> 1 sequence per core). All weights
replicated, stored in DRAM as fp16 (halves HBM traffic; TRN2 PE runs fp16 at
the same 1 cycle/row as f32r). Activations are kept feature-major
[D=partitions, T=free]; the residual stream stays fp32, while the normalized
stream / attention operands / GEMM inputs are fp16 (PSUM accumulation is fp32).

LayerNorm stats are broadcast-reduced with a ones[128,128] matmul so the whole
mean/rsqrt chain runs on [128,S] tiles and never round-trips the PE for a
separate broadcast. Attention computes transposed scores [kt, qt]; sigmoid is
one activation per chunk with an in-place causal-mask multiply on the diagonal
block; the AV matmuls for the two heads of a pair write one PSUM tile at
partition offsets 0/64 (col-tiled, concurrent on the PE array).

Host side only marshals: embedding gather + positional add, weight pre-tiling
into DMA-contiguous fp16 layouts, and the final [V,T] -> [S,V] untranspose.
"""

import sys

sys.path.insert(0, "/opt/trn_rl_repo")

import numpy as np
import ml_dtypes

import concourse.bass as bass  # noqa: F401  (keeps bass registered before bacc)
import concourse.tile as tile
from concourse import bacc, mybir
from concourse.bass import ts
from concourse.bass_utils import run_bass_kernel_spmd

B, S, D, H, L, V = 8, 512, 1024, 16, 6, 32000
DH = D // H
LN_EPS = 1e-5
N_CORES = 8
NC_D = D // 128      # 8 feature chunks
NC_T = S // 128      # 4 token chunks
NC_V = V // 128      # 250 vocab chunks
NP = 8               # head pairs

F32 = mybir.dt.float32
F32R = mybir.dt.float32r
F16 = mybir.dt.float16
AF = mybir.ActivationFunctionType
OP = mybir.AluOpType
NPF16 = np.float16

_prog_cache = {}


def _build(cfg):
    """Build + compile the SPMD per-core program. cfg is a hashable tuple."""
    (use_lng, use_lnb, use_bqk, use_bv, use_bg, use_bo, use_bp, rpb_nz) = cfg

    nc = bacc.Bacc("TRN2", target_bir_lowering=False, debug=False)

    x0_d = nc.dram_tensor("x0t", [NC_D, 128, S], F32R, kind="ExternalInput").ap()
    x0b_d = nc.dram_tensor("x0b", [NC_D, 128, S], F16, kind="ExternalInput").ap()
    wqk_d = nc.dram_tensor("wqk", [L, 16, 128, 1024], F16, kind="ExternalInput").ap()
    wv_d = nc.dram_tensor("wv", [L, 16, 128, 512], F16, kind="ExternalInput").ap()
    wg_d = nc.dram_tensor("wg", [L, NC_D, 128, 1024], F16, kind="ExternalInput").ap()
    wo_d = nc.dram_tensor("wo", [L, NC_D, 128, 1024], F16, kind="ExternalInput").ap()
    wp_d = nc.dram_tensor("wp", [NC_V, 128, 1024], F16, kind="ExternalInput").ap()
    tri_d = nc.dram_tensor("tri", [128, 128], F16, kind="ExternalInput").ap()
    ones_d = nc.dram_tensor("onesq", [128, 128], F16, kind="ExternalInput").ap()
    need_oner = rpb_nz or use_bv
    oner_d = nc.dram_tensor("oner", [1, 128], F32R, kind="ExternalInput").ap() if need_oner else None
    lng_d = nc.dram_tensor("lng", [L, NC_D, 128], F32, kind="ExternalInput").ap() if use_lng else None
    lnb_d = nc.dram_tensor("lnb", [L, NC_D, 128], F32, kind="ExternalInput").ap() if use_lnb else None
    bqk_d = nc.dram_tensor("bqk", [L, 16, 128], F32, kind="ExternalInput").ap() if use_bqk else None
    bv_d = nc.dram_tensor("bv", [L, 2, 1, 512], F32R, kind="ExternalInput").ap() if use_bv else None
    bg_d = nc.dram_tensor("bg", [L, NC_D, 128], F32, kind="ExternalInput").ap() if use_bg else None
    bo_d = nc.dram_tensor("bo", [L, NC_D, 128], F32, kind="ExternalInput").ap() if use_bo else None
    bp_d = nc.dram_tensor("bp", [NC_V, 128], F32, kind="ExternalInput").ap() if use_bp else None
    rpb_d = nc.dram_tensor("rpb", [1, H * L], F32R, kind="ExternalInput").ap() if rpb_nz else None
    out_d = nc.dram_tensor("logits_t", [NC_V, 128, S], F16, kind="ExternalOutput").ap()

    with tile.TileContext(nc) as tc, nc.allow_low_precision(
        reason="fp16 GEMM operands; accumulation stays fp32 in PSUM"
    ):
        from contextlib import ExitStack

        with ExitStack() as ctx:
            cp = ctx.enter_context(tc.tile_pool(name="consts", bufs=1))
            xp = ctx.enter_context(tc.tile_pool(name="x", bufs=2))
            xbp = ctx.enter_context(tc.tile_pool(name="xbf", bufs=2))
            xnp = ctx.enter_context(tc.tile_pool(name="xn", bufs=2))
            up = ctx.enter_context(tc.tile_pool(name="u", bufs=1))
            vp = ctx.enter_context(tc.tile_pool(name="v", bufs=1))
            gp = ctx.enter_context(tc.tile_pool(name="g", bufs=1))
            qkp = ctx.enter_context(tc.tile_pool(name="qk", bufs=3))
            tmp = ctx.enter_context(tc.tile_pool(name="tmp", bufs=3))
            bp_pool = ctx.enter_context(tc.tile_pool(name="bcast", bufs=2))
            atp = ctx.enter_context(tc.tile_pool(name="at", bufs=4))
            wbp = ctx.enter_context(tc.tile_pool(name="wb", bufs=8))
            wvp = ctx.enter_context(tc.tile_pool(name="wvp", bufs=2))
            op_pool = ctx.enter_context(tc.tile_pool(name="out", bufs=4))
            prm = ctx.enter_context(tc.tile_pool(name="prm", bufs=2))
            pmm = ctx.enter_context(tc.tile_pool(name="pmm", bufs=2, space="PSUM"))
            pao = ctx.enter_context(tc.tile_pool(name="pao", bufs=2, space="PSUM"))
            psc = ctx.enter_context(tc.tile_pool(name="psc", bufs=4, space="PSUM"))

            mm = nc.tensor.matmul

            ones_t = cp.tile([128, 128], F16)
            nc.sync.dma_start(ones_t[:], ones_d[:])
            tri_t = cp.tile([128, 128], F16)
            nc.sync.dma_start(tri_t[:], tri_d[:])
            eps_t = cp.tile([128, 1], F32)
            nc.vector.memset(eps_t[:], LN_EPS)
            if need_oner:
                oner = cp.tile([1, 128], F32R)
                nc.sync.dma_start(oner[:], oner_d[:])
            if rpb_nz:
                rpb_row = cp.tile([1, H * L], F32R)
                nc.sync.dma_start(rpb_row[:], rpb_d[:])
                # broadcast to [128, H*L] so column slices give per-partition bias
                prb = psc.tile([128, 512], F32, tag="sc")
                mm(prb[:, : H * L], oner[:], rpb_row[:], start=True, stop=True)
                rpb_t = cp.tile([128, H * L], F32)
                nc.scalar.copy(rpb_t[:], prb[:, : H * L])
            if use_bp:
                bp_t = cp.tile([128, NC_V], F32)
                nc.sync.dma_start(bp_t[:], bp_d.rearrange("v p -> p v"))

            x_cur = xp.tile([128, NC_D * S], F32R, tag="x")
            xb_cur = xbp.tile([128, NC_D * S], F16, tag="xb")
            for c in range(NC_D):
                nc.sync.dma_start(xb_cur[:, ts(c, S)], x0b_d[c])
            for c in range(NC_D):
                # separate DMA queue: the fp32 residual isn't needed until the
                # first out-projection, so keep it off the weight-load queue
                nc.gpsimd.dma_start(x_cur[:, ts(c, S)], x0_d[c])

            for l in range(L):
                # ---- per-layer params ----
                if use_lng:
                    lng_t = prm.tile([128, NC_D], F32, tag="lng")
                    nc.sync.dma_start(lng_t[:], lng_d[l].rearrange("c p -> p c"))
                if use_lnb:
                    lnb_t = prm.tile([128, NC_D], F32, tag="lnb")
                    nc.sync.dma_start(lnb_t[:], lnb_d[l].rearrange("c p -> p c"))
                if use_bqk:
                    bqk_t = prm.tile([128, 16], F32, tag="bqk")
                    nc.sync.dma_start(bqk_t[:], bqk_d[l].rearrange("c p -> p c"))
                if use_bg:
                    bg_t = prm.tile([128, NC_D], F32, tag="bg")
                    nc.sync.dma_start(bg_t[:], bg_d[l].rearrange("c p -> p c"))
                if use_bo:
                    bo_t = prm.tile([128, NC_D], F32, tag="bo")
                    nc.sync.dma_start(bo_t[:], bo_d[l].rearrange("c p -> p c"))

                # ---- LayerNorm stats from the fp16 shadow, broadcast-reduced
                # to [128, S]; the whole chain runs on fp16 tiles (DVE 2x) ----
                ps_s = psc.tile([128, S], F32, tag="sc")
                ps_q = psc.tile([128, S], F32, tag="sc")
                for c in range(NC_D):
                    xc = xb_cur[:, ts(c, S)]
                    mm(ps_s[:], ones_t[:], xc, start=(c == 0), stop=(c == NC_D - 1))
                    sq = tmp.tile([128, S], F16, tag="sqb")
                    nc.vector.tensor_mul(sq[:], xc, xc)
                    mm(ps_q[:], ones_t[:], sq[:], start=(c == 0), stop=(c == NC_D - 1))
                mu_b = bp_pool.tile([128, S], F16, tag="mu")
                nc.vector.tensor_scalar_mul(mu_b[:], ps_s[:], 1.0 / D)
                musq = tmp.tile([128, S], F16, tag="sqb")
                nc.vector.tensor_mul(musq[:], mu_b[:], mu_b[:])
                var = tmp.tile([128, S], F32, tag="tmp")
                nc.vector.scalar_tensor_tensor(
                    var[:], ps_q[:], 1.0 / D, musq[:], op0=OP.mult, op1=OP.subtract
                )
                rs_b = bp_pool.tile([128, S], F16, tag="rs")
                nc.scalar.activation(rs_b[:], var[:], AF.Abs_reciprocal_sqrt,
                                     bias=eps_t[:, 0:1], scale=1.0)

                xn = xnp.tile([128, NC_D * S], F16, tag="xn")
                for c in range(NC_D):
                    t = tmp.tile([128, S], F16, tag="sqb")
                    nc.vector.tensor_sub(t[:], xb_cur[:, ts(c, S)], mu_b[:])
                    if use_lng or use_lnb:
                        t2 = tmp.tile([128, S], F32, tag="tmp")
                        nc.vector.tensor_mul(t2[:], t[:], rs_b[:])
                        nc.scalar.activation(
                            xn[:, ts(c, S)], t2[:], AF.Identity,
                            bias=(lnb_t[:, c : c + 1] if use_lnb else 0.0),
                            scale=(lng_t[:, c : c + 1] if use_lng else 1.0),
                        )
                    else:
                        nc.vector.tensor_mul(xn[:, ts(c, S)], t[:], rs_b[:])

                # ---- qk projection for pairs 0/1, four PSUM banks wide: each
                # xn chunk unlocks 4 matmuls, so the PE streams through the
                # LayerNorm chunk-production window without stalling ----
                def qk_copy(dst, pq, fidx):
                    if use_bqk:
                        nc.scalar.activation(
                            dst[:], pq[:], AF.Identity,
                            bias=bqk_t[:, fidx : fidx + 1], scale=1.0,
                        )
                    else:
                        nc.vector.tensor_copy(dst[:], pq[:])

                qk4_f = (0, 8, 1, 9)  # q0, k0, q1, k1
                qk4_w = []
                for fidx in qk4_f:
                    wt = wbp.tile([128, 1024], F16, tag="wb")
                    nc.sync.dma_start(wt[:], wqk_d[l, fidx])
                    qk4_w.append(wt)
                qk4_p = [pmm.tile([128, S], F32, tag="mm", name="qk4p0"),
                         pmm.tile([128, S], F32, tag="mm", name="qk4p1"),
                         pao.tile([128, S], F32, tag="ao", name="qk4p2"),
                         pao.tile([128, S], F32, tag="ao", name="qk4p3")]
                for fi in range(NC_D):
                    for j in range(4):
                        mm(qk4_p[j][:], qk4_w[j][:, ts(fi, 128)], xn[:, ts(fi, S)],
                           start=(fi == 0), stop=(fi == NC_D - 1))
                qk01 = []
                for j, fidx in enumerate(qk4_f):
                    dst = qkp.tile([128, S], F16, tag=("qc" if fidx < 8 else "kc"))
                    qk_copy(dst, qk4_p[j], fidx)
                    qk01.append(dst)

                # ---- V projection, token-major [t, fo] (xn stationary, wv moving) ----
                wv_t = wvp.tile([128, 16 * 512], F16, tag="wv")
                for i in range(16):
                    nc.sync.dma_start(wv_t[:, ts(i, 512)], wv_d[l, i])
                if use_bv:
                    vb = []
                    for foB in range(2):
                        pvb = psc.tile([128, 512], F32, tag="sc")
                        bvrow = cp.tile([1, 512], F32R, tag=f"bvr{foB}")
                        nc.sync.dma_start(bvrow[:], bv_d[l, foB])
                        mm(pvb[:], oner[:], bvrow[:], start=True, stop=True)
                        vbt = bp_pool.tile([128, 512], F32, tag="vb")
                        nc.scalar.copy(vbt[:], pvb[:])
                        vb.append(vbt)
                v = vp.tile([128, NC_T * 1024], F16, tag="v")
                for foB in range(2):
                    for tc_ in range(NC_T):
                        pv = pmm.tile([128, 512], F32, tag="mm")
                        for fi in range(NC_D):
                            mm(
                                pv[:],
                                xn[:, fi * S + tc_ * 128 : fi * S + tc_ * 128 + 128],
                                wv_t[:, ts(foB * 8 + fi, 512)],
                                start=(fi == 0),
                                stop=(fi == NC_D - 1),
                            )
                        dst = v[:, tc_ * 1024 + foB * 512 : tc_ * 1024 + foB * 512 + 512]
                        if use_bv:
                            nc.vector.tensor_add(dst, pv[:], vb[foB][:])
                        else:
                            nc.vector.tensor_copy(dst, pv[:])

                # ---- gate: u = silu(xn @ gate_w + bg), computed as
                # sigmoid(z)*z so ScalarE only ever needs the SIGMOID table
                # during the layer body (no SILU<->SIGMOID table ping-pong) ----
                u = up.tile([128, NC_D * S], F16, tag="u")

                def gate_chunk(f, pu):
                    us = tmp.tile([128, S], F16, tag="sqb", name=f"us{l}_{f}")
                    nc.scalar.activation(
                        us[:], pu[:], AF.Sigmoid,
                        bias=(bg_t[:, f : f + 1] if use_bg else 0.0), scale=1.0,
                    )
                    if use_bg:
                        ub = tmp.tile([128, S], F32, tag="tmp", name=f"ub{l}_{f}")
                        nc.scalar.activation(ub[:], pu[:], AF.Identity,
                                             bias=bg_t[:, f : f + 1], scale=1.0)
                        nc.vector.tensor_mul(u[:, ts(f, S)], ub[:], us[:])
                    else:
                        nc.vector.tensor_mul(u[:, ts(f, S)], pu[:], us[:])

                for f in range(NC_D - 2):
                    wt = wbp.tile([128, 1024], F16, tag="wb")
                    nc.sync.dma_start(wt[:], wg_d[l, f])
                    pu = pmm.tile([128, S], F32, tag="mm")
                    for fi in range(NC_D):
                        mm(pu[:], wt[:, ts(fi, 128)], xn[:, ts(fi, S)],
                           start=(fi == 0), stop=(fi == NC_D - 1))
                    gate_chunk(f, pu)

                # ---- attention; the remaining qk projections (pairs 2..7) and
                # gate chunks 6/7 are queued as filler matmul thunks that the
                # attention loop interleaves between score/AV chains, so the PE
                # streams through the sigmoid handoffs without stalling. ----
                g = gp.tile([128, NC_D * S], F16, tag="g")
                fillers = []

                def queue_qk(p_):
                    qc = qkp.tile([128, S], F16, tag="qc", name=f"qc{l}_{p_}")
                    kc = qkp.tile([128, S], F16, tag="kc", name=f"kc{l}_{p_}")
                    for (dst, fidx) in ((qc, p_), (kc, 8 + p_)):
                        wt = wbp.tile([128, 1024], F16, tag="wb", name=f"wqk{l}_{fidx}")
                        nc.sync.dma_start(wt[:], wqk_d[l, fidx])
                        state = {}

                        def emit(fi, wt=wt, dst=dst, fidx=fidx, state=state):
                            if fi == 0:
                                state["pq"] = pmm.tile([128, S], F32, tag="mm",
                                                       name=f"pq{l}_{fidx}")
                            mm(state["pq"][:], wt[:, ts(fi, 128)], xn[:, ts(fi, S)],
                               start=(fi == 0), stop=(fi == NC_D - 1))
                            if fi == NC_D - 1:
                                qk_copy(dst, state["pq"], fidx)

                        for fi in range(NC_D):
                            fillers.append(lambda fi=fi, emit=emit: emit(fi))
                    return qc, kc

                def queue_gate(f):
                    wt = wbp.tile([128, 1024], F16, tag="wb", name=f"wg{l}_{f}")
                    nc.sync.dma_start(wt[:], wg_d[l, f])
                    state = {}

                    def emit(fi, wt=wt, f=f, state=state):
                        if fi == 0:
                            state["pu"] = pmm.tile([128, S], F32, tag="mm",
                                                   name=f"pu{l}_{f}")
                        mm(state["pu"][:], wt[:, ts(fi, 128)], xn[:, ts(fi, S)],
                           start=(fi == 0), stop=(fi == NC_D - 1))
                        if fi == NC_D - 1:
                            gate_chunk(f, state["pu"])

                    for fi in range(NC_D):
                        fillers.append(lambda fi=fi, emit=emit: emit(fi))

                def attention(p_, qc, kc, fill_quota):
                    # ao: one [128, S] PSUM tile; the two heads of the pair
                    # write partitions 0:64 / 64:128 (col-tiled, concurrent).
                    ao = pao.tile([128, S], F32, tag="ao")

                    # software-pipeline: scores for chunk c+1 are issued before
                    # the AV matmuls of chunk c, so the PE streams through the
                    # sigmoid/mask handoff without stalling.
                    def make_att(c):
                        n = S - 128 * c
                        atts = []
                        for (o, hh) in ((0, 2 * p_), (64, 2 * p_ + 1)):
                            sc = psc.tile([128, S], F32, tag="sc")
                            mm(sc[:, :n], kc[o : o + 64, ts(c, 128)],
                               qc[o : o + 64, c * 128 : S], start=True, stop=True)
                            sig_bias = rpb_t[:, l * H + hh : l * H + hh + 1] if rpb_nz else 0.0
                            att = atp.tile([128, S], F16, tag="at")
                            nc.scalar.activation(att[:, :n], sc[:, :n], AF.Sigmoid,
                                                 bias=sig_bias, scale=DH**-0.5)
                            nc.vector.tensor_mul(att[:, 0:128], att[:, 0:128], tri_t[:])
                            atts.append(att)
                        return atts

                    def do_av(c, atts):
                        n = S - 128 * c
                        for (o, att) in ((0, atts[0]), (64, atts[1])):
                            # two interleaved accumulation groups in one bank,
                            # partition-disjoint: start's pending-zero marking
                            # is scoped to the partitions each matmul writes.
                            mm(
                                ao[o : o + 64, c * 128 : S],
                                v[:, c * 1024 + p_ * 128 + o : c * 1024 + p_ * 128 + o + 64],
                                att[:, 0:n],
                                start=(c == 0),
                                stop=(c == NC_T - 1),
                                skip_group_check=True,
                            )

                    def fill(k):
                        for _ in range(min(k, len(fillers))):
                            fillers.pop(0)()

                    per_step = fill_quota // NC_T
                    att_prev = make_att(0)
                    fill(per_step)
                    for c in range(1, NC_T):
                        atts = make_att(c)
                        fill(per_step)
                        do_av(c - 1, att_prev)
                        att_prev = atts
                    do_av(NC_T - 1, att_prev)

                    nc.vector.tensor_mul(g[:, ts(p_, S)], ao[:], u[:, ts(p_, S)])

                # Filler FIFO order + per-pair quotas satisfy the deadlines:
                # qk(p) fully drained before attention(p) issues its scores;
                # gate chunk f drained before pair f's g-mul.
                qks = {0: (qk01[0], qk01[1]), 1: (qk01[2], qk01[3])}
                for p2 in range(2, 7):
                    qks[p2] = queue_qk(p2)
                queue_gate(NC_D - 2)
                qks[7] = queue_qk(7)
                queue_gate(NC_D - 1)
                quotas = [16, 16, 16, 16, 16, 16, 8, 8]
                for p_ in range(NP):
                    attention(p_, *qks.pop(p_), fill_quota=quotas[p_])
                assert not fillers, f"{len(fillers)} filler thunks left"

                # ---- out projection + residual (fp32) + bf16 shadow ----
                last = l == L - 1
                x_new = None if last else xp.tile([128, NC_D * S], F32R, tag="x")
                xb_new = xbp.tile([128, NC_D * S], F16, tag="xb")
                for f in range(NC_D):
                    wt = wbp.tile([128, 1024], F16, tag="wb")
                    nc.sync.dma_start(wt[:], wo_d[l, f])
                    pd = pmm.tile([128, S], F32, tag="mm")
                    for fi in range(NC_D):
                        mm(pd[:], wt[:, ts(fi, 128)], g[:, ts(fi, S)],
                           start=(fi == 0), stop=(fi == NC_D - 1))
                    src = pd[:]
                    if use_bo:
                        t3 = tmp.tile([128, S], F32, tag="tmp")
                        nc.scalar.activation(t3[:], pd[:], AF.Identity,
                                             bias=bo_t[:, f : f + 1], scale=1.0)
                        src = t3[:]
                    if last:
                        nc.vector.tensor_add(xb_new[:, ts(f, S)], src, x_cur[:, ts(f, S)])
                    else:
                        nc.vector.tensor_add(x_new[:, ts(f, S)], src, x_cur[:, ts(f, S)])
                        nc.vector.tensor_copy(xb_new[:, ts(f, S)], x_new[:, ts(f, S)])
                x_cur = x_new
                xb_cur = xb_new

            # ---- logits: [V, T] feature(vocab)-major; x is fp16 after layer L-1 ----
            for vo in range(NC_V):
                wt = wbp.tile([128, 1024], F16, tag="wb")
                nc.sync.dma_start(wt[:], wp_d[vo])
                pl = pmm.tile([128, S], F32, tag="mm")
                for fi in range(NC_D):
                    mm(pl[:], wt[:, ts(fi, 128)], xb_cur[:, ts(fi, S)],
                       start=(fi == 0), stop=(fi == NC_D - 1))
                ot = op_pool.tile([128, S], F16, tag="o")
                if use_bp:
                    nc.scalar.activation(ot[:], pl[:], AF.Identity,
                                         bias=bp_t[:, vo : vo + 1], scale=1.0)
                elif vo % 2 == 0:
                    nc.scalar.copy(ot[:], pl[:])
                else:
                    nc.vector.tensor_copy(ot[:], pl[:])
                nc.sync.dma_start(out_d[vo], ot[:])

    nc.compile()
    return nc


def _get_program(cfg):
    nc = _prog_cache.get(cfg)
    if nc is None:
        nc = _build(cfg)
        _prog_cache[cfg] = nc
    return nc


def _marshal(inputs):
    """Host-side input marshalling into DMA-friendly layouts."""
    f = np.float32
    input_ids = np.asarray(inputs["input_ids"])
    emb = np.asarray(inputs["embedding"], f)
    pos = np.asarray(inputs["pos_encoding"], f)
    qkv_w = np.asarray(inputs["qkv_w"], f)
    gate_w = np.asarray(inputs["gate_w"], f)
    out_w = np.asarray(inputs["out_w"], f)
    proj_w = np.asarray(inputs["proj_w"], f)

    x0 = emb[input_ids] + pos[:, :S, :]                       # [B, S, D]
    # feature-major per core: [D, S] -> [NC_D, 128, S]
    x0t = np.ascontiguousarray(x0.transpose(0, 2, 1)).reshape(B, NC_D, 128, S)

    # lhsT tiles [K=fi(128), M=fo(128)] packed 8-fi-wide: [l, fo, 128p, 8c*128j]
    def lhs_tiles(w, nfo):  # w: [L, D, nfo*128]
        r = w.reshape(L, NC_D, 128, nfo, 128).transpose(0, 3, 2, 1, 4)
        return np.ascontiguousarray(r.reshape(L, nfo, 128, NC_D * 128).astype(NPF16))

    wqk = lhs_tiles(qkv_w[:, :, :2048], 16)                   # [6,16,128,1024]
    wg = lhs_tiles(gate_w, NC_D)                              # [6,8,128,1024]
    wo = lhs_tiles(out_w, NC_D)                               # [6,8,128,1024]
    # v-section as moving tiles [K=fi(128), N=fo(512)]: [l, foB*8+fi, 128, 512]
    wv = qkv_w[:, :, 2048:].reshape(L, NC_D, 128, 2, 512).transpose(0, 3, 1, 2, 4)
    wv = np.ascontiguousarray(wv.reshape(L, 16, 128, 512).astype(NPF16))
    wp = proj_w.reshape(NC_D, 128, NC_V, 128).transpose(2, 1, 0, 3)
    wp = np.ascontiguousarray(wp.reshape(NC_V, 128, NC_D * 128).astype(NPF16))

    tri = np.triu(np.ones((128, 128), f)).astype(NPF16)
    onesq = np.ones((128, 128), NPF16)
    oner = np.ones((1, 128), f)

    qkv_b = np.asarray(inputs["qkv_b"], f)
    gate_b = np.asarray(inputs["gate_b"], f)
    out_b = np.asarray(inputs["out_b"], f)
    proj_b = np.asarray(inputs["proj_b"], f)
    ln_g = np.asarray(inputs["ln_g"], f)
    ln_b = np.asarray(inputs["ln_b"], f)
    rpb = np.asarray(inputs["rel_pos_bias"], f)

    use_lng = not np.all(ln_g == 1.0)
    use_lnb = np.any(ln_b != 0.0)
    use_bqk = np.any(qkv_b[:, :2048] != 0.0)
    use_bv = np.any(qkv_b[:, 2048:] != 0.0)
    use_bg = np.any(gate_b != 0.0)
    use_bo = np.any(out_b != 0.0)
    use_bp = np.any(proj_b != 0.0)
    rpb_nz = bool(np.any(rpb != 0.0))

    shared = {
        "wqk": wqk, "wv": wv, "wg": wg, "wo": wo, "wp": wp,
        "tri": tri, "onesq": onesq,
    }
    if rpb_nz or use_bv:
        shared["oner"] = oner
    if use_lng:
        shared["lng"] = np.ascontiguousarray(ln_g.reshape(L, NC_D, 128))
    if use_lnb:
        shared["lnb"] = np.ascontiguousarray(ln_b.reshape(L, NC_D, 128))
    if use_bqk:
        shared["bqk"] = np.ascontiguousarray(qkv_b[:, :2048].reshape(L, 16, 128))
    if use_bv:
        shared["bv"] = np.ascontiguousarray(qkv_b[:, 2048:].reshape(L, 2, 1, 512))
    if use_bg:
        shared["bg"] = np.ascontiguousarray(gate_b.reshape(L, NC_D, 128))
    if use_bo:
        shared["bo"] = np.ascontiguousarray(out_b.reshape(L, NC_D, 128))
    if use_bp:
        shared["bp"] = np.ascontiguousarray(proj_b.reshape(NC_V, 128))
    if rpb_nz:
        shared["rpb"] = np.ascontiguousarray(rpb.reshape(1, L * H))

    cfg = (use_lng, use_lnb, use_bqk, use_bv, use_bg, use_bo, use_bp, rpb_nz)
    in_maps = []
    for b in range(B):
        m = dict(shared)
        m["x0t"] = np.ascontiguousarray(x0t[b])
        m["x0b"] = np.ascontiguousarray(x0t[b].astype(NPF16))
        in_maps.append(m)
    return cfg, in_maps


def run(inputs, mm_mode="bf16", trace=False):
    cfg, in_maps = _marshal(inputs)
    nc = _get_program(cfg)
    res = run_bass_kernel_spmd(nc, in_maps, core_ids=list(range(N_CORES)), trace=trace)
    out = np.empty((B, S, V), np.float32)
    for b in range(B):
        lt = res.results[b]["logits_t"].astype(np.float32).reshape(V, S)
        out[b] = lt.T
    return out, res


def kernel(**inputs) -> np.ndarray:
    out, _ = run(inputs)
    return out
